# revision 6
# baseline (speedup 1.0000x reference)
"""Trainium2 Bass kernel for DenseContrastiveLoss.

Math (see reference):
  q  = l2norm(df1.reshape(3136, 128))        (rows)
  k  = l2norm(df2.reshape(3136, 128))
  sim[b] = q[b] @ k[b].T per batch of 196    -> pos_sim = rowmax(sim)/T
           (value at the argmax == max row value, so no gather needed)
  queue_n = l2norm(queue)                    [65536, 128]
  neg_sim = (q @ queue_n.T)/T                [3136, 65536]
  loss = mean(logsumexp([pos, neg_row]) - pos)

Sharding: queue dim split across 8 cores (8192 rows each). Every core
computes the full q/k prologue (cheap) and its neg_sim column slab
[3136, 8192] plus per-row sum(exp(neg)) partials. Host combines the
8 partials + pos_sim into the scalar loss (tiny) and concatenates the
slabs.

Scaling trick: q rows are scaled by 1/(||q||*T) so both the sim matmul
and the neg matmul come out already divided by TEMPERATURE.
"""

import numpy as np

B, N, D = 16, 196, 128
R = B * N                 # 3136 query rows
RP = 3200                 # padded to 25 * 128
MT = RP // 128            # 25 row tiles
Q = 65536
NCORES = 8
QS = Q // NCORES          # 8192 queue rows per core
QT = QS // 128            # 64 queue tiles per core
INV_T = 5.0               # 1 / TEMPERATURE

_CACHE = {}


def _emit(nc, tc, tens, parts=frozenset({"mm", "copy", "act", "dma", "sim"})):
    import concourse.mybir as mybir
    from concourse import masks

    f32 = mybir.dt.float32
    X = mybir.AxisListType.X
    mult = mybir.AluOpType.mult
    add = mybir.AluOpType.add
    amax = mybir.AluOpType.max
    Exp = mybir.ActivationFunctionType.Exp
    Sqrt = mybir.ActivationFunctionType.Sqrt

    df1, df2, qsh, neg_out, pos_out, se_out = tens

    with tc.tile_pool(name="persist", bufs=1) as persist:
        qT = persist.tile([128, RP], f32, tag="qT")
        kT = persist.tile([128, RP], f32, tag="kT")
        quT = persist.tile([128, QS], f32, tag="quT")
        coll = persist.tile([128, MT], f32, tag="coll")
        ident = persist.tile([128, 128], f32, tag="ident")
        masks.make_identity(nc, ident[:])
        nc.vector.memset(coll[:], 0.0)

        with (
            tc.tile_pool(name="pro", bufs=1) as pro,
            tc.tile_pool(name="small", bufs=1) as small,
            tc.tile_pool(name="pro_ps", bufs=4, space="PSUM") as pps,
        ):
            q_raw = pro.tile([128, RP], f32, tag="qraw")
            k_raw = pro.tile([128, RP], f32, tag="kraw")
            u_raw = pro.tile([128, QS], f32, tag="uraw")
            sq = pro.tile([128, QS], f32, tag="sq")

            nc.sync.dma_start(
                out=q_raw[:].rearrange("p (m d) -> p m d", d=D),
                in_=df1[:, :].rearrange("(m p) d -> p m d", p=128),
            )
            nc.sync.dma_start(
                out=k_raw[:].rearrange("p (m d) -> p m d", d=D),
                in_=df2[:, :].rearrange("(m p) d -> p m d", p=128),
            )
            nc.sync.dma_start(
                out=u_raw[:].rearrange("p (m d) -> p m d", d=D),
                in_=qsh[:, :].rearrange("(m p) d -> p m d", p=128),
            )

            # Row sum-of-squares for each 128-wide d-group.
            ssq_q = small.tile([128, MT], f32, tag="ssq_q")
            ssq_k = small.tile([128, MT], f32, tag="ssq_k")
            ssq_u = small.tile([128, QT], f32, tag="ssq_u")
            nc.vector.tensor_tensor(sq[:, :RP], q_raw[:], q_raw[:], op=mult)
            nc.vector.tensor_reduce(
                ssq_q[:], sq[:, :RP].rearrange("p (m d) -> p m d", d=D), X, add
            )
            nc.vector.tensor_tensor(sq[:, :RP], k_raw[:], k_raw[:], op=mult)
            nc.vector.tensor_reduce(
                ssq_k[:], sq[:, :RP].rearrange("p (m d) -> p m d", d=D), X, add
            )
            nc.vector.tensor_tensor(sq[:], u_raw[:], u_raw[:], op=mult)
            nc.vector.tensor_reduce(
                ssq_u[:], sq[:].rearrange("p (m d) -> p m d", d=D), X, add
            )

            # inv = 1/sqrt(ssq), one Newton step to clean up ACT sqrt.
            def rsqrt(inv_tile, s_tile, n):
                nrm = small.tile([128, QT], f32, tag="nrm")
                t = small.tile([128, QT], f32, tag="nt")
                nc.scalar.activation(nrm[:, :n], s_tile[:, :n], Sqrt)
                nc.vector.reciprocal(inv_tile[:, :n], nrm[:, :n])
                nc.vector.tensor_tensor(
                    t[:, :n], inv_tile[:, :n], inv_tile[:, :n], op=mult
                )
                nc.vector.tensor_tensor(t[:, :n], t[:, :n], s_tile[:, :n], op=mult)
                nc.vector.tensor_scalar(
                    out=t[:, :n], in0=t[:, :n],
                    scalar1=-0.5, scalar2=1.5, op0=mult, op1=add,
                )
                nc.vector.tensor_tensor(
                    inv_tile[:, :n], inv_tile[:, :n], t[:, :n], op=mult
                )

            inv_q = small.tile([128, MT], f32, tag="inv_q")
            inv_k = small.tile([128, MT], f32, tag="inv_k")
            inv_u = small.tile([128, QT], f32, tag="inv_u")
            rsqrt(inv_q, ssq_q, MT)
            rsqrt(inv_k, ssq_k, MT)
            rsqrt(inv_u, ssq_u, QT)

            # Scale rows in place, then transpose tiles into qT/kT/quT.
            for m in range(MT):
                c = m * 128
                nc.vector.tensor_scalar(
                    out=q_raw[:, c : c + 128], in0=q_raw[:, c : c + 128],
                    scalar1=inv_q[:, m : m + 1], scalar2=INV_T,
                    op0=mult, op1=mult,
                )
                tp = pps.tile([128, 196], f32, tag="pp")
                nc.tensor.transpose(tp[:, :128], q_raw[:, c : c + 128], ident[:])
                nc.vector.tensor_copy(qT[:, c : c + 128], tp[:, :128])
            for m in range(MT):
                c = m * 128
                nc.vector.tensor_scalar_mul(
                    k_raw[:, c : c + 128], k_raw[:, c : c + 128],
                    inv_k[:, m : m + 1],
                )
                tp = pps.tile([128, 196], f32, tag="pp")
                nc.tensor.transpose(tp[:, :128], k_raw[:, c : c + 128], ident[:])
                nc.vector.tensor_copy(kT[:, c : c + 128], tp[:, :128])
            for t_ in range(QT):
                c = t_ * 128
                nc.vector.tensor_scalar_mul(
                    u_raw[:, c : c + 128], u_raw[:, c : c + 128],
                    inv_u[:, t_ : t_ + 1],
                )
                tp = pps.tile([128, 196], f32, tag="pp")
                nc.tensor.transpose(tp[:, :128], u_raw[:, c : c + 128], ident[:])
                nc.vector.tensor_copy(quT[:, c : c + 128], tp[:, :128])

            # pos_sim: per-batch sim matmul + row max.
            for b in range(B) if "sim" in parts else []:
                for off in (0, 128):
                    m_ = min(128, N - off)
                    r0 = b * N + off
                    ps = pps.tile([128, 196], f32, tag="pp")
                    nc.tensor.matmul(
                        ps[:m_, :N],
                        lhsT=qT[:, r0 : r0 + m_],
                        rhs=kT[:, b * N : (b + 1) * N],
                        start=True, stop=True,
                    )
                    pm = small.tile([128, 1], f32, tag="pm")
                    nc.vector.tensor_reduce(pm[:m_, :1], ps[:m_, :N], X, amax)
                    nc.sync.dma_start(
                        out=pos_out[r0 : r0 + m_, 0:1], in_=pm[:m_, :1]
                    )

        # Main loop: neg_sim slab, 25 row-tiles x 16 matmuls of 512.
        with (
            tc.tile_pool(name="osb_pool", bufs=2) as opool,
            tc.tile_pool(name="exp_pool", bufs=1) as epool,
            tc.tile_pool(name="mm_ps", bufs=2, space="PSUM") as mpool,
        ):
            exp_scr = epool.tile([128, QS], f32, tag="exp")
            if parts != frozenset({"mm", "copy", "act", "dma", "sim"}):
                nc.vector.memset(exp_scr[:], 1.0)
            for m in range(MT):
                m_ = 128 if m < MT - 1 else R - (MT - 1) * 128
                osb = opool.tile([128, QS], f32, tag="osb")
                for g in range(4):
                    ps = mpool.tile([128, 2048], f32, tag="mps")
                    if "mm" in parts:
                        for j in range(4):
                            c0 = (g * 4 + j) * 512
                            nc.tensor.matmul(
                                ps[:m_, j * 512 : (j + 1) * 512],
                                lhsT=qT[:, m * 128 : m * 128 + m_],
                                rhs=quT[:, c0 : c0 + 512],
                                start=True, stop=True,
                            )
                    if "copy" in parts:
                        src_ap = (
                            ps[:m_, :2048]
                            if "mm" in parts
                            else exp_scr[:m_, g * 2048 : (g + 1) * 2048]
                        )
                        nc.vector.tensor_copy(
                            osb[:m_, g * 2048 : (g + 1) * 2048], src_ap
                        )
                if "act" in parts:
                    nc.scalar.activation(
                        exp_scr[:m_, :], osb[:m_, :], Exp,
                        accum_out=coll[:m_, m : m + 1],
                    )
                if "dma" in parts:
                    dma_src = osb if "copy" in parts else exp_scr
                    eng = nc.sync if m % 2 == 0 else nc.scalar
                    eng.dma_start(
                        out=neg_out[m * 128 : m * 128 + m_, :], in_=dma_src[:m_, :]
                    )

            nc.sync.dma_start(out=se_out[:, :], in_=coll[:])


def _build(reps=1, parts=frozenset({"mm", "copy", "act", "dma", "sim"})):
    key = ("nc", reps, tuple(sorted(parts)))
    if key in _CACHE:
        return _CACHE[key]

    import concourse.bacc as bacc
    import concourse.tile as tile
    import concourse.mybir as mybir

    f32 = mybir.dt.float32
    nc = bacc.Bacc("TRN2", target_bir_lowering=False, debug=False)

    df1 = nc.dram_tensor("df1", [RP, D], f32, kind="ExternalInput")
    df2 = nc.dram_tensor("df2", [RP, D], f32, kind="ExternalInput")
    qsh = nc.dram_tensor("qsh", [QS, D], f32, kind="ExternalInput")
    neg_out = nc.dram_tensor("neg_out", [R, QS], f32, kind="ExternalOutput")
    pos_out = nc.dram_tensor("pos_out", [R, 1], f32, kind="ExternalOutput")
    se_out = nc.dram_tensor("se_out", [128, MT], f32, kind="ExternalOutput")
    tens = (df1, df2, qsh, neg_out, pos_out, se_out)

    with tile.TileContext(nc) as tc:
        for _ in range(reps):
            _emit(nc, tc, tens, parts)

    nc.compile()
    _CACHE[key] = nc
    return nc


def _run(df1_p, df2_p, queue, trace=False, reps=1):
    from concourse.bass_utils import run_bass_kernel_spmd

    nc = _build(reps)
    in_maps = [
        {
            "df1": df1_p,
            "df2": df2_p,
            "qsh": np.ascontiguousarray(queue[c * QS : (c + 1) * QS]),
        }
        for c in range(NCORES)
    ]
    return run_bass_kernel_spmd(
        nc, in_maps, core_ids=list(range(NCORES)), trace=trace
    )


def kernel(
    dense_features_1,
    dense_features_2,
    backbone_features_1=None,
    backbone_features_2=None,
    queue=None,
    _trace=False,
    _return_res=False,
):
    df1 = np.asarray(dense_features_1, dtype=np.float32).reshape(R, D)
    df2 = np.asarray(dense_features_2, dtype=np.float32).reshape(R, D)
    queue = np.asarray(queue, dtype=np.float32)

    pad = np.ones((RP - R, D), dtype=np.float32)
    df1_p = np.concatenate([df1, pad], axis=0)
    df2_p = np.concatenate([df2, pad], axis=0)

    res = _run(df1_p, df2_p, queue, trace=_trace)

    neg_sim = np.concatenate(
        [res.results[c]["neg_out"] for c in range(NCORES)], axis=1
    )
    pos_flat = res.results[0]["pos_out"][:, 0].astype(np.float32)
    sumexp = np.zeros(R, dtype=np.float64)
    for c in range(NCORES):
        sumexp += res.results[c]["se_out"].T.reshape(RP)[:R].astype(np.float64)
    pos64 = pos_flat.astype(np.float64)
    loss = np.float32(np.mean(np.log(np.exp(pos64) + sumexp) - pos64))

    out = (loss, pos_flat.reshape(B, N), neg_sim)
    if _return_res:
        return out, res
    return out


# revision 8
# speedup vs baseline: 2.0950x; 2.0950x over previous
"""Trainium2 Bass kernel for DenseContrastiveLoss.

Math (see reference):
  q  = l2norm(df1.reshape(3136, 128))        (rows)
  k  = l2norm(df2.reshape(3136, 128))
  sim[b] = q[b] @ k[b].T per batch of 196    -> pos_sim = rowmax(sim)/T
           (value at the argmax == max row value, so no gather needed)
  queue_n = l2norm(queue)                    [65536, 128]
  neg_sim = (q @ queue_n.T)/T                [3136, 65536]
  loss = mean(logsumexp([pos, neg_row]) - pos)

Sharding: queue dim split across 8 cores (8192 rows each). Every core
computes the full q/k prologue (cheap) and its neg_sim column slab
[3136, 8192] plus per-row sum(exp(neg)) partials. Host combines the
8 partials + pos_sim into the scalar loss (tiny) and concatenates the
slabs.

Scaling trick: q rows are scaled by 1/(||q||*T) so both the sim matmul
and the neg matmul come out already divided by TEMPERATURE.
"""

import numpy as np

B, N, D = 16, 196, 128
R = B * N                 # 3136 query rows
RP = 3200                 # padded to 25 * 128
MT = RP // 128            # 25 row tiles
Q = 65536
NCORES = 8
QS = Q // NCORES          # 8192 queue rows per core
QT = QS // 128            # 64 queue tiles per core
INV_T = 5.0               # 1 / TEMPERATURE

_CACHE = {}


def _emit(nc, tc, tens, parts=frozenset({"mm", "copy", "act", "dma", "sim"})):
    import concourse.mybir as mybir
    from concourse import masks

    f32 = mybir.dt.float32
    X = mybir.AxisListType.X
    mult = mybir.AluOpType.mult
    add = mybir.AluOpType.add
    amax = mybir.AluOpType.max
    Exp = mybir.ActivationFunctionType.Exp
    Sqrt = mybir.ActivationFunctionType.Sqrt
    Square = mybir.ActivationFunctionType.Square

    df1, df2, qsh, neg_out, pos_out, se_out = tens

    with (
        tc.tile_pool(name="persist", bufs=1) as persist,
        tc.tile_pool(name="small", bufs=1) as small,
    ):
        qT = persist.tile([128, RP], f32, tag="qT")
        kT = persist.tile([128, RP], f32, tag="kT")
        quT = persist.tile([128, QS], f32, tag="quT")
        coll = persist.tile([128, MT], f32, tag="coll")
        pos_c = persist.tile([128, 2 * B], f32, tag="pos_c")
        ident = persist.tile([128, 128], f32, tag="ident")
        masks.make_identity(nc, ident[:])
        nc.vector.memset(coll[:], 0.0)

        ssq_q = small.tile([128, MT], f32, tag="ssq_q")
        ssq_k = small.tile([128, MT], f32, tag="ssq_k")
        ssq_u = small.tile([128, QT], f32, tag="ssq_u")
        inv_q = small.tile([128, MT], f32, tag="inv_q")
        inv_k = small.tile([128, MT], f32, tag="inv_k")
        inv_u = small.tile([128, QT], f32, tag="inv_u")

        # inv = 1/sqrt(ssq), one Newton step to clean up ACT sqrt.
        def rsqrt(inv_ap, s_ap):
            n = s_ap.shape[-1]
            nrm = small.tile([128, 16], f32, tag="nrm")
            t = small.tile([128, 16], f32, tag="nt")
            nc.scalar.activation(nrm[:, :n], s_ap, Sqrt)
            nc.vector.reciprocal(inv_ap, nrm[:, :n])
            nc.vector.tensor_tensor(t[:, :n], inv_ap, inv_ap, op=mult)
            nc.vector.tensor_tensor(t[:, :n], t[:, :n], s_ap, op=mult)
            nc.vector.tensor_scalar(
                out=t[:, :n], in0=t[:, :n],
                scalar1=-0.5, scalar2=1.5, op0=mult, op1=add,
            )
            nc.vector.tensor_tensor(inv_ap, inv_ap, t[:, :n], op=mult)

        with (
            tc.tile_pool(name="pro", bufs=1) as pro,
            tc.tile_pool(name="sqs", bufs=4) as sqs,
            tc.tile_pool(name="pro_ps", bufs=4, space="PSUM") as pps,
        ):
            q_raw = pro.tile([128, RP], f32, tag="qraw")
            k_raw = pro.tile([128, RP], f32, tag="kraw")
            u_raw = pro.tile([128, QS], f32, tag="uraw")

            # Input loads, split across both HWDGE rings.
            nc.sync.dma_start(
                out=u_raw[:, : QS // 2].rearrange("p (m d) -> p m d", d=D),
                in_=qsh[: QS // 2, :].rearrange("(m p) d -> p m d", p=128),
            )
            nc.scalar.dma_start(
                out=u_raw[:, QS // 2 :].rearrange("p (m d) -> p m d", d=D),
                in_=qsh[QS // 2 :, :].rearrange("(m p) d -> p m d", p=128),
            )
            nc.sync.dma_start(
                out=q_raw[:].rearrange("p (m d) -> p m d", d=D),
                in_=df1[:, :].rearrange("(m p) d -> p m d", p=128),
            )
            nc.scalar.dma_start(
                out=k_raw[:].rearrange("p (m d) -> p m d", d=D),
                in_=df2[:, :].rearrange("(m p) d -> p m d", p=128),
            )

            def norm_transpose(raw, ssq, inv, dst, ntiles, chunk, extra_scale):
                """ACT square+accum per tile -> rsqrt per chunk -> scale ->
                PE transpose -> DVE copy into dst, pipelined in chunks."""
                for c0 in range(0, ntiles, chunk):
                    c1 = min(c0 + chunk, ntiles)
                    for i in range(c0, c1):
                        scr = sqs.tile([128, 128], f32, tag="sqscr")
                        nc.scalar.activation(
                            scr[:], raw[:, i * 128 : (i + 1) * 128], Square,
                            accum_out=ssq[:, i : i + 1],
                        )
                    rsqrt(inv[:, c0:c1], ssq[:, c0:c1])
                    for i in range(c0, c1):
                        cc = i * 128
                        if extra_scale is None:
                            nc.vector.tensor_scalar_mul(
                                raw[:, cc : cc + 128], raw[:, cc : cc + 128],
                                inv[:, i : i + 1],
                            )
                        else:
                            nc.vector.tensor_scalar(
                                out=raw[:, cc : cc + 128],
                                in0=raw[:, cc : cc + 128],
                                scalar1=inv[:, i : i + 1], scalar2=extra_scale,
                                op0=mult, op1=mult,
                            )
                        tp = pps.tile([128, 128], f32, tag="pp")
                        nc.tensor.transpose(
                            tp[:], raw[:, cc : cc + 128], ident[:]
                        )
                        nc.vector.tensor_copy(dst[:, cc : cc + 128], tp[:])

            norm_transpose(u_raw, ssq_u, inv_u, quT, QT, 16, None)
            norm_transpose(q_raw, ssq_q, inv_q, qT, MT, 13, INV_T)
            norm_transpose(k_raw, ssq_k, inv_k, kT, MT, 13, None)

        # Main loop: neg_sim slab, 25 row-tiles x 16 matmuls of 512.
        with (
            tc.tile_pool(name="osb_pool", bufs=3) as opool,
            tc.tile_pool(name="exp_pool", bufs=1) as epool,
            tc.tile_pool(name="mm_ps", bufs=2, space="PSUM") as mpool,
        ):
            exp_scr = epool.tile([128, QS], f32, tag="exp")
            if "copy" not in parts or "mm" not in parts:
                nc.vector.memset(exp_scr[:], 1.0)
            for m in range(MT):
                m_ = 128 if m < MT - 1 else R - (MT - 1) * 128
                osb = opool.tile([128, QS], f32, tag="osb")
                for g in range(4):
                    ps = mpool.tile([128, 2048], f32, tag="mps")
                    if "mm" in parts:
                        for j in range(4):
                            c0 = (g * 4 + j) * 512
                            nc.tensor.matmul(
                                ps[:m_, j * 512 : (j + 1) * 512],
                                lhsT=qT[:, m * 128 : m * 128 + m_],
                                rhs=quT[:, c0 : c0 + 512],
                                start=True, stop=True,
                            )
                    if "copy" in parts:
                        src_ap = (
                            ps[:m_, :2048]
                            if "mm" in parts
                            else exp_scr[:m_, g * 2048 : (g + 1) * 2048]
                        )
                        nc.vector.tensor_copy(
                            osb[:m_, g * 2048 : (g + 1) * 2048], src_ap
                        )
                if "act" in parts:
                    nc.scalar.activation(
                        exp_scr[:m_, :], osb[:m_, :], Exp,
                        accum_out=coll[:m_, m : m + 1],
                    )
                if "dma" in parts:
                    dma_src = osb if "copy" in parts else exp_scr
                    eng = nc.sync if m % 2 == 0 else nc.scalar
                    eng.dma_start(
                        out=neg_out[m * 128 : m * 128 + m_, :], in_=dma_src[:m_, :]
                    )

        # Tail: per-batch sim matmuls + row max (own small PSUM pool).
        if "sim" in parts:
            with tc.tile_pool(name="sim_ps", bufs=4, space="PSUM") as sps:
                for b in range(B):
                    for off in (0, 128):
                        m_ = min(128, N - off)
                        r0 = b * N + off
                        idx = 2 * b + (1 if off else 0)
                        ps = sps.tile([128, 196], f32, tag="sp")
                        nc.tensor.matmul(
                            ps[:m_, :N],
                            lhsT=qT[:, r0 : r0 + m_],
                            rhs=kT[:, b * N : (b + 1) * N],
                            start=True, stop=True,
                        )
                        nc.vector.tensor_reduce(
                            pos_c[:m_, idx : idx + 1], ps[:m_, :N], X, amax
                        )
            nc.sync.dma_start(out=pos_out[:, :], in_=pos_c[:])

        nc.sync.dma_start(out=se_out[:, :], in_=coll[:])


def _build(reps=1, parts=frozenset({"mm", "copy", "act", "dma", "sim"})):
    key = ("nc", reps, tuple(sorted(parts)))
    if key in _CACHE:
        return _CACHE[key]

    import concourse.bacc as bacc
    import concourse.tile as tile
    import concourse.mybir as mybir

    f32 = mybir.dt.float32
    nc = bacc.Bacc("TRN2", target_bir_lowering=False, debug=False)

    df1 = nc.dram_tensor("df1", [RP, D], f32, kind="ExternalInput")
    df2 = nc.dram_tensor("df2", [RP, D], f32, kind="ExternalInput")
    qsh = nc.dram_tensor("qsh", [QS, D], f32, kind="ExternalInput")
    neg_out = nc.dram_tensor("neg_out", [R, QS], f32, kind="ExternalOutput")
    pos_out = nc.dram_tensor("pos_out", [128, 2 * B], f32, kind="ExternalOutput")
    se_out = nc.dram_tensor("se_out", [128, MT], f32, kind="ExternalOutput")
    tens = (df1, df2, qsh, neg_out, pos_out, se_out)

    with tile.TileContext(nc) as tc:
        for _ in range(reps):
            _emit(nc, tc, tens, parts)

    nc.compile()
    _CACHE[key] = nc
    return nc


def _run(df1_p, df2_p, queue, trace=False, reps=1):
    from concourse.bass_utils import run_bass_kernel_spmd

    nc = _build(reps)
    in_maps = [
        {
            "df1": df1_p,
            "df2": df2_p,
            "qsh": np.ascontiguousarray(queue[c * QS : (c + 1) * QS]),
        }
        for c in range(NCORES)
    ]
    return run_bass_kernel_spmd(
        nc, in_maps, core_ids=list(range(NCORES)), trace=trace
    )


def kernel(
    dense_features_1,
    dense_features_2,
    backbone_features_1=None,
    backbone_features_2=None,
    queue=None,
    _trace=False,
    _return_res=False,
):
    df1 = np.asarray(dense_features_1, dtype=np.float32).reshape(R, D)
    df2 = np.asarray(dense_features_2, dtype=np.float32).reshape(R, D)
    queue = np.asarray(queue, dtype=np.float32)

    pad = np.ones((RP - R, D), dtype=np.float32)
    df1_p = np.concatenate([df1, pad], axis=0)
    df2_p = np.concatenate([df2, pad], axis=0)

    res = _run(df1_p, df2_p, queue, trace=_trace)

    neg_sim = np.concatenate(
        [res.results[c]["neg_out"] for c in range(NCORES)], axis=1
    )
    pos_grid = res.results[0]["pos_out"]
    pos_flat = np.empty(R, dtype=np.float32)
    for b in range(B):
        for off, idx in ((0, 2 * b), (128, 2 * b + 1)):
            m_ = min(128, N - off)
            r0 = b * N + off
            pos_flat[r0 : r0 + m_] = pos_grid[:m_, idx]
    sumexp = np.zeros(R, dtype=np.float64)
    for c in range(NCORES):
        sumexp += res.results[c]["se_out"].T.reshape(RP)[:R].astype(np.float64)
    pos64 = pos_flat.astype(np.float64)
    loss = np.float32(np.mean(np.log(np.exp(pos64) + sumexp) - pos64))

    out = (loss, pos_flat.reshape(B, N), neg_sim)
    if _return_res:
        return out, res
    return out
